# revision 3
# baseline (speedup 1.0000x reference)
"""Trainium2 Bass kernel for the ConvIntrinsic GNN message-passing problem.

Shapes (hardcoded): B=1, N=50000, R=5, A=8, F=16, T=32, O=8.

Strategy:
  - Shard vertices across 8 NeuronCores (6250 each, padded to 6272 = 49*128).
  - Algebraic fold (host, tiny weights only): kernel (R,A,R,A), rotated
    neighbor weights and self weights collapse into one linear map
    W_big[(xy,f) -> (o,t)] of shape [640+16, 256] applied per vertex to the
    barycentric-interpolated patch signal, plus a bias and ReLU.
  - The mesh signal table (50000 x 16) is stored bf16 in SBUF transposed,
    one feature per partition-channel, consecutive-row PAIRS along the free
    axis -> full table is gatherable by GPSIMD ap_gather with int16 pair
    indices in a single call; parity selection is folded into the
    barycentric weights (6 weight terms per (vertex, xy) instead of 3).
  - GPSIMD groups (16 partitions each) own 5 of the 40 (r,a) cells for all
    vertices; gather output is j-major [3, 5*TV, 2] per partition so the
    weighted reduce is a mul + two contiguous adds + one strided pair-add
    on DVE (no expensive tensor_reduce).
  - Per 128-vertex tile: 1 ap_gather (1920 idx/group), 4 DVE ops, 12
    PSUM-accumulated matmuls [128x128]x[128x128], one ACT bias+ReLU
    (bias is identical for both 128-column halves since 128 % T == 0),
    one combined output DMA. Triple buffering to keep all engines busy.
  - Output written [128, 2, Vpad] per core; host transposes/assembles.
"""

import sys

sys.path.insert(0, "/opt/trn_rl_repo")

import numpy as np
import ml_dtypes

from concourse import bacc, tile
import concourse.mybir as mybir
from concourse.bass_utils import run_bass_kernel_spmd

BF16 = ml_dtypes.bfloat16

N = 50000
F = 16
RR = 5
A = 8
T = 32
O = 8
NC = 8

TV = 128                    # vertices per round (tile)
VS = N // NC                # 6250 vertices per core
ROUNDS = (VS + TV - 1) // TV  # 49
VNC = ROUNDS * TV           # 6272 padded vertices per core
XY = RR * A                 # 40
XYB = XY // 8               # 5 xy cells per gpsimd group
IT = TV * XYB * 3           # 1920 gather indices per group per round
MT = TV * XYB               # 640 interp columns per group per round
E2 = N // 2                 # 25000 row pairs
WB = 3                      # work pool depth (pipelining)


def _build_program():
    nc = bacc.Bacc("TRN2", target_bir_lowering=False, debug=False)
    f32 = mybir.dt.float32
    bf16 = mybir.dt.bfloat16
    i16 = mybir.dt.int16

    d_table = nc.dram_tensor("table", [128, 2 * E2], bf16, kind="ExternalInput")
    d_idx = nc.dram_tensor("idx", [128, ROUNDS * (IT // 16)], i16, kind="ExternalInput")
    d_wexp = nc.dram_tensor("wexp", [128, ROUNDS * IT * 2], bf16, kind="ExternalInput")
    d_msh = nc.dram_tensor("meshtp", [128, VNC], bf16, kind="ExternalInput")
    d_wst = nc.dram_tensor("wstat", [128, 12 * 128], bf16, kind="ExternalInput")
    d_bias = nc.dram_tensor("biasv", [128, 1], f32, kind="ExternalInput")
    d_out = nc.dram_tensor("out", [128, 2, VNC], f32, kind="ExternalOutput")

    with nc.allow_low_precision("bf16 gather/interp pipeline"):
        with tile.TileContext(nc) as tc:
            with (
                tc.tile_pool(name="const", bufs=1) as cp,
                tc.tile_pool(name="work", bufs=WB) as wp,
                tc.tile_pool(name="psum", bufs=4, space="PSUM") as pp,
            ):
                sb_tbl = cp.tile([128, 2 * E2], bf16)
                sb_idx = cp.tile([128, ROUNDS * (IT // 16)], i16)
                sb_msh = cp.tile([128, VNC], bf16)
                sb_wst = cp.tile([128, 12 * 128], bf16)
                sb_bias = cp.tile([128, 1], f32)

                nc.sync.dma_start(sb_tbl[:], d_table[:])
                nc.sync.dma_start(sb_idx[:], d_idx[:])
                nc.sync.dma_start(sb_msh[:], d_msh[:])
                nc.sync.dma_start(sb_wst[:], d_wst[:])
                nc.sync.dma_start(sb_bias[:], d_bias[:])

                for r in range(ROUNDS):
                    wexp_t = wp.tile([128, 3, MT, 2], mybir.dt.bfloat16,
                                     name="wexp_t", tag="wexp")
                    gath_t = wp.tile([128, 3, MT, 2], mybir.dt.bfloat16,
                                     name="gath_t", tag="gath")
                    acc_t = wp.tile([128, MT, 2], mybir.dt.bfloat16,
                                    name="acc_t", tag="acc")
                    interp_t = wp.tile([128, MT], mybir.dt.bfloat16,
                                       name="interp_t", tag="interp")
                    out_t = wp.tile([128, 2, TV], f32, name="out_t", tag="outt")

                    nc.sync.dma_start(
                        wexp_t[:], d_wexp[:, r * IT * 2 : (r + 1) * IT * 2]
                    )
                    nc.gpsimd.ap_gather(
                        gath_t[:],
                        sb_tbl[:],
                        sb_idx[:, r * (IT // 16) : (r + 1) * (IT // 16)],
                        channels=128,
                        num_elems=E2,
                        d=2,
                        num_idxs=IT,
                    )
                    nc.vector.tensor_mul(gath_t[:], gath_t[:], wexp_t[:])
                    nc.vector.tensor_add(acc_t[:], gath_t[:, 0], gath_t[:, 1])
                    nc.vector.tensor_add(acc_t[:], acc_t[:], gath_t[:, 2])
                    nc.vector.tensor_add(
                        interp_t[:], acc_t[:, :, 0], acc_t[:, :, 1]
                    )
                    ps = pp.tile([128, 2 * TV], f32, name="ps", tag="ps")
                    for h in range(2):
                        for kt in range(6):
                            if kt < 5:
                                mov = interp_t[:, kt * TV : (kt + 1) * TV]
                            else:
                                mov = sb_msh[:, r * TV : (r + 1) * TV]
                            nc.tensor.matmul(
                                ps[:, h * TV : (h + 1) * TV],
                                sb_wst[:, (kt * 2 + h) * 128 : (kt * 2 + h + 1) * 128],
                                mov,
                                start=(kt == 0),
                                stop=(kt == 5),
                            )
                    nc.scalar.activation(
                        out_t[:],
                        ps[:],
                        mybir.ActivationFunctionType.Relu,
                        bias=sb_bias[:, 0:1],
                    )
                    nc.sync.dma_start(
                        d_out[:, :, r * TV : (r + 1) * TV], out_t[:]
                    )

    nc.compile()
    return nc


def _host_prep(mesh_signal, bary_coordinates, neighbor_weights, self_weights, bias, kernel):
    """Builds the per-core input maps. Only weight folding (tiny tensors) and
    layout/sharding transforms of the big inputs happen here."""
    mesh = np.asarray(mesh_signal)[0]          # [N, F] f32
    bary = np.asarray(bary_coordinates)[0]     # [N, R, A, 3, 2]
    nw = np.asarray(neighbor_weights)          # [T, R, A, F]
    sw = np.asarray(self_weights)              # [T, 1, F]
    bs = np.asarray(bias)                      # [T]
    ker = np.asarray(kernel)                   # [R, A, R, A]

    # ---- weight fold: W_big[(x*8+y)*16+f, o*32+t] ----
    wrot = np.stack([np.roll(nw, -o, axis=2) for o in range(O)])  # [O,T,R,A,F]
    w_big = np.einsum("raxy,otraf->xyfot", ker, wrot).reshape(XY * F, O * T)

    # stationary tiles [128, 12*128]: p = 16*g + f
    wst = np.zeros((128, 12 * 128), dtype=np.float32)
    p = np.arange(128)
    g = p // 16
    f = p % 16
    for kt in range(5):
        xy = g * XYB + kt
        rows = w_big[xy * F + f]               # [128, 256]
        for h in range(2):
            wst[:, (kt * 2 + h) * 128 : (kt * 2 + h + 1) * 128] = rows[
                :, h * 128 : (h + 1) * 128
            ]
    # center tile: self_weights[t, 0, f] at partitions p<16, broadcast over o
    ot = np.arange(O * T)
    cen = np.zeros((128, O * T), dtype=np.float32)
    cen[:F, :] = sw[ot % T, 0, :].T            # [F, 256]
    for h in range(2):
        wst[:, (5 * 2 + h) * 128 : (5 * 2 + h + 1) * 128] = cen[:, h * 128 : (h + 1) * 128]
    wst = wst.astype(BF16)

    # bias per ot-row: t = (h*128 + p) % 32 == p % 32 for both halves
    biasv = np.ascontiguousarray(bs[np.arange(128) % T][:, None].astype(np.float32))

    # ---- table: [128, 2*E2] bf16, one feature per channel, row pairs ----
    tbl16 = np.ascontiguousarray(mesh.T.astype(BF16))    # [16, N]
    table = np.tile(tbl16, (8, 1))                       # [128, N]

    idx_all = bary[..., 0].astype(np.int32).reshape(N, XY, 3)
    w_all = bary[..., 1].astype(np.float32).reshape(N, XY, 3)

    in_maps = []
    for s in range(NC):
        vs, ve = s * VS, (s + 1) * VS
        idx = np.zeros((VNC, XY, 3), dtype=np.int32)
        w = np.zeros((VNC, XY, 3), dtype=np.float32)
        idx[:VS] = idx_all[vs:ve]
        w[:VS] = w_all[vs:ve]

        par = (idx & 1).astype(np.float32)
        pairi = (idx >> 1).astype(np.int16)

        # index order per (group, round): (j, kt, v)  [j-major]
        pr = pairi.reshape(ROUNDS, TV, 8, XYB, 3)
        pro = pr.transpose(2, 0, 4, 3, 1).reshape(8, ROUNDS, IT)
        idx_in = (
            pro.reshape(8, ROUNDS, IT // 16, 16)
            .transpose(0, 3, 1, 2)
            .reshape(128, ROUNDS * (IT // 16))
        )
        idx_in = np.ascontiguousarray(idx_in)

        # parity-folded weights to match gather layout [j, kt, v, par]
        k2 = np.arange(2, dtype=np.float32)
        we6 = w[..., None] * (par[..., None] == k2)      # [VNC, XY, 3, 2]
        wr = we6.reshape(ROUNDS, TV, 8, XYB, 3, 2)
        wro = wr.transpose(2, 0, 4, 3, 1, 5).reshape(8, ROUNDS * IT * 2)
        wexp_in = np.ascontiguousarray(
            np.repeat(wro.astype(BF16), 16, axis=0)
        )  # [128, ROUNDS*IT*2]

        msh = np.zeros((128, VNC), dtype=BF16)
        msh[:F, :VS] = tbl16[:, vs:ve]

        in_maps.append(
            dict(
                table=table,
                idx=idx_in,
                wexp=wexp_in,
                meshtp=msh,
                wstat=wst,
                biasv=biasv,
            )
        )
    return in_maps


_PROGRAM_CACHE = {}


def _get_program():
    if "nc" not in _PROGRAM_CACHE:
        _PROGRAM_CACHE["nc"] = _build_program()
    return _PROGRAM_CACHE["nc"]


def kernel(mesh_signal, bary_coordinates, neighbor_weights, self_weights, bias, kernel,
           _trace=False, _core_ids=None):
    nc = _get_program()
    in_maps = _host_prep(
        mesh_signal, bary_coordinates, neighbor_weights, self_weights, bias, kernel
    )
    core_ids = list(range(NC)) if _core_ids is None else _core_ids
    res = run_bass_kernel_spmd(nc, in_maps[: len(core_ids)], core_ids, trace=_trace)
    out = np.zeros((1, N, O, T), dtype=np.float32)
    for i in range(len(core_ids)):
        o = res.results[i]["out"]              # [128, 2, VNC]
        out[0, i * VS : (i + 1) * VS] = (
            o[:, :, :VS].transpose(2, 1, 0).reshape(VS, O, T)
        )
    if _trace:
        globals()["kernel"]._last_exec_ns = res.exec_time_ns
    return out


# revision 4
# speedup vs baseline: 1.0094x; 1.0094x over previous
"""Trainium2 Bass kernel for the ConvIntrinsic GNN message-passing problem.

Shapes (hardcoded): B=1, N=50000, R=5, A=8, F=16, T=32, O=8.

Strategy:
  - Shard vertices across 8 NeuronCores (6250 each, padded to 6272 = 49*128).
  - Algebraic fold (host, tiny weights only): kernel (R,A,R,A), rotated
    neighbor weights and self weights collapse into one linear map
    W_big[(xy,f) -> (o,t)] of shape [640+16, 256] applied per vertex to the
    barycentric-interpolated patch signal, plus a bias and ReLU.
  - The mesh signal table (50000 x 16) is stored bf16 in SBUF transposed,
    one feature per partition-channel, consecutive-row PAIRS along the free
    axis -> full table is gatherable by GPSIMD ap_gather with int16 pair
    indices in a single call; parity selection is folded into the
    barycentric weights (6 weight terms per (vertex, xy) instead of 3).
  - GPSIMD groups (16 partitions each) own 5 of the 40 (r,a) cells for all
    vertices; gather output is j-major [3, 5*TV, 2] per partition so the
    weighted reduce is a mul + two contiguous adds + one strided pair-add
    on DVE (no expensive tensor_reduce).
  - Per 128-vertex tile: 1 ap_gather (1920 idx/group), 4 DVE ops, 12
    PSUM-accumulated matmuls [128x128]x[128x128], one ACT bias+ReLU
    (bias is identical for both 128-column halves since 128 % T == 0),
    one combined output DMA. Triple buffering to keep all engines busy.
  - Output written [128, 2, Vpad] per core; host transposes/assembles.
"""

import sys

sys.path.insert(0, "/opt/trn_rl_repo")

import numpy as np
import ml_dtypes

from concourse import bacc, tile
import concourse.mybir as mybir
from concourse.bass_utils import run_bass_kernel_spmd

BF16 = ml_dtypes.bfloat16

N = 50000
F = 16
RR = 5
A = 8
T = 32
O = 8
NC = 8

TV = 192                    # vertices per round (tile)
VS = N // NC                # 6250 vertices per core
ROUNDS = (VS + TV - 1) // TV  # 49
VNC = ROUNDS * TV           # 6272 padded vertices per core
XY = RR * A                 # 40
XYB = XY // 8               # 5 xy cells per gpsimd group
IT = TV * XYB * 3           # 1920 gather indices per group per round
MT = TV * XYB               # 640 interp columns per group per round
E2 = N // 2                 # 25000 row pairs
WB = 2                      # work pool depth (pipelining)


def _build_program():
    nc = bacc.Bacc("TRN2", target_bir_lowering=False, debug=False)
    f32 = mybir.dt.float32
    bf16 = mybir.dt.bfloat16
    i16 = mybir.dt.int16

    d_table = nc.dram_tensor("table", [128, 2 * E2], bf16, kind="ExternalInput")
    d_idx = nc.dram_tensor("idx", [128, ROUNDS * (IT // 16)], i16, kind="ExternalInput")
    d_wexp = nc.dram_tensor("wexp", [128, ROUNDS * IT * 2], bf16, kind="ExternalInput")
    d_msh = nc.dram_tensor("meshtp", [128, VNC], bf16, kind="ExternalInput")
    d_wst = nc.dram_tensor("wstat", [128, 12 * 128], bf16, kind="ExternalInput")
    d_bias = nc.dram_tensor("biasv", [128, 1], f32, kind="ExternalInput")
    d_out = nc.dram_tensor("out", [128, 2, VNC], f32, kind="ExternalOutput")

    with nc.allow_low_precision("bf16 gather/interp pipeline"):
        with tile.TileContext(nc) as tc:
            with (
                tc.tile_pool(name="const", bufs=1) as cp,
                tc.tile_pool(name="work", bufs=WB) as wp,
                tc.tile_pool(name="psum", bufs=4, space="PSUM") as pp,
            ):
                sb_tbl = cp.tile([128, 2 * E2], bf16)
                sb_idx = cp.tile([128, ROUNDS * (IT // 16)], i16)
                sb_msh = cp.tile([128, VNC], bf16)
                sb_wst = cp.tile([128, 12 * 128], bf16)
                sb_bias = cp.tile([128, 1], f32)

                nc.sync.dma_start(sb_tbl[:], d_table[:])
                nc.sync.dma_start(sb_idx[:], d_idx[:])
                nc.sync.dma_start(sb_msh[:], d_msh[:])
                nc.sync.dma_start(sb_wst[:], d_wst[:])
                nc.sync.dma_start(sb_bias[:], d_bias[:])

                for r in range(ROUNDS):
                    wexp_t = wp.tile([128, 3, MT, 2], mybir.dt.bfloat16,
                                     name="wexp_t", tag="wexp")
                    gath_t = wp.tile([128, 3, MT, 2], mybir.dt.bfloat16,
                                     name="gath_t", tag="gath")
                    acc_t = wp.tile([128, MT, 2], mybir.dt.bfloat16,
                                    name="acc_t", tag="acc")
                    interp_t = wp.tile([128, MT], mybir.dt.bfloat16,
                                       name="interp_t", tag="interp")
                    out_t = wp.tile([128, 2, TV], f32, name="out_t", tag="outt")

                    nc.sync.dma_start(
                        wexp_t[:], d_wexp[:, r * IT * 2 : (r + 1) * IT * 2]
                    )
                    nc.gpsimd.ap_gather(
                        gath_t[:],
                        sb_tbl[:],
                        sb_idx[:, r * (IT // 16) : (r + 1) * (IT // 16)],
                        channels=128,
                        num_elems=E2,
                        d=2,
                        num_idxs=IT,
                    )
                    nc.vector.tensor_mul(gath_t[:], gath_t[:], wexp_t[:])
                    nc.vector.tensor_add(acc_t[:], gath_t[:, 0], gath_t[:, 1])
                    nc.vector.tensor_add(acc_t[:], acc_t[:], gath_t[:, 2])
                    nc.vector.tensor_add(
                        interp_t[:], acc_t[:, :, 0], acc_t[:, :, 1]
                    )
                    ps = pp.tile([128, 2 * TV], f32, name="ps", tag="ps")
                    for h in range(2):
                        for kt in range(6):
                            if kt < 5:
                                mov = interp_t[:, kt * TV : (kt + 1) * TV]
                            else:
                                mov = sb_msh[:, r * TV : (r + 1) * TV]
                            nc.tensor.matmul(
                                ps[:, h * TV : (h + 1) * TV],
                                sb_wst[:, (kt * 2 + h) * 128 : (kt * 2 + h + 1) * 128],
                                mov,
                                start=(kt == 0),
                                stop=(kt == 5),
                            )
                    nc.scalar.activation(
                        out_t[:],
                        ps[:],
                        mybir.ActivationFunctionType.Relu,
                        bias=sb_bias[:, 0:1],
                    )
                    nc.sync.dma_start(
                        d_out[:, :, r * TV : (r + 1) * TV], out_t[:]
                    )

    nc.compile()
    return nc


def _host_prep(mesh_signal, bary_coordinates, neighbor_weights, self_weights, bias, kernel):
    """Builds the per-core input maps. Only weight folding (tiny tensors) and
    layout/sharding transforms of the big inputs happen here."""
    mesh = np.asarray(mesh_signal)[0]          # [N, F] f32
    bary = np.asarray(bary_coordinates)[0]     # [N, R, A, 3, 2]
    nw = np.asarray(neighbor_weights)          # [T, R, A, F]
    sw = np.asarray(self_weights)              # [T, 1, F]
    bs = np.asarray(bias)                      # [T]
    ker = np.asarray(kernel)                   # [R, A, R, A]

    # ---- weight fold: W_big[(x*8+y)*16+f, o*32+t] ----
    wrot = np.stack([np.roll(nw, -o, axis=2) for o in range(O)])  # [O,T,R,A,F]
    w_big = np.einsum("raxy,otraf->xyfot", ker, wrot).reshape(XY * F, O * T)

    # stationary tiles [128, 12*128]: p = 16*g + f
    wst = np.zeros((128, 12 * 128), dtype=np.float32)
    p = np.arange(128)
    g = p // 16
    f = p % 16
    for kt in range(5):
        xy = g * XYB + kt
        rows = w_big[xy * F + f]               # [128, 256]
        for h in range(2):
            wst[:, (kt * 2 + h) * 128 : (kt * 2 + h + 1) * 128] = rows[
                :, h * 128 : (h + 1) * 128
            ]
    # center tile: self_weights[t, 0, f] at partitions p<16, broadcast over o
    ot = np.arange(O * T)
    cen = np.zeros((128, O * T), dtype=np.float32)
    cen[:F, :] = sw[ot % T, 0, :].T            # [F, 256]
    for h in range(2):
        wst[:, (5 * 2 + h) * 128 : (5 * 2 + h + 1) * 128] = cen[:, h * 128 : (h + 1) * 128]
    wst = wst.astype(BF16)

    # bias per ot-row: t = (h*128 + p) % 32 == p % 32 for both halves
    biasv = np.ascontiguousarray(bs[np.arange(128) % T][:, None].astype(np.float32))

    # ---- table: [128, 2*E2] bf16, one feature per channel, row pairs ----
    tbl16 = np.ascontiguousarray(mesh.T.astype(BF16))    # [16, N]
    table = np.tile(tbl16, (8, 1))                       # [128, N]

    idx_all = bary[..., 0].astype(np.int32).reshape(N, XY, 3)
    w_all = bary[..., 1].astype(np.float32).reshape(N, XY, 3)

    in_maps = []
    for s in range(NC):
        vs, ve = s * VS, (s + 1) * VS
        idx = np.zeros((VNC, XY, 3), dtype=np.int32)
        w = np.zeros((VNC, XY, 3), dtype=np.float32)
        idx[:VS] = idx_all[vs:ve]
        w[:VS] = w_all[vs:ve]

        par = (idx & 1).astype(np.float32)
        pairi = (idx >> 1).astype(np.int16)

        # index order per (group, round): (j, kt, v)  [j-major]
        pr = pairi.reshape(ROUNDS, TV, 8, XYB, 3)
        pro = pr.transpose(2, 0, 4, 3, 1).reshape(8, ROUNDS, IT)
        idx_in = (
            pro.reshape(8, ROUNDS, IT // 16, 16)
            .transpose(0, 3, 1, 2)
            .reshape(128, ROUNDS * (IT // 16))
        )
        idx_in = np.ascontiguousarray(idx_in)

        # parity-folded weights to match gather layout [j, kt, v, par]
        k2 = np.arange(2, dtype=np.float32)
        we6 = w[..., None] * (par[..., None] == k2)      # [VNC, XY, 3, 2]
        wr = we6.reshape(ROUNDS, TV, 8, XYB, 3, 2)
        wro = wr.transpose(2, 0, 4, 3, 1, 5).reshape(8, ROUNDS * IT * 2)
        wexp_in = np.ascontiguousarray(
            np.repeat(wro.astype(BF16), 16, axis=0)
        )  # [128, ROUNDS*IT*2]

        msh = np.zeros((128, VNC), dtype=BF16)
        msh[:F, :VS] = tbl16[:, vs:ve]

        in_maps.append(
            dict(
                table=table,
                idx=idx_in,
                wexp=wexp_in,
                meshtp=msh,
                wstat=wst,
                biasv=biasv,
            )
        )
    return in_maps


_PROGRAM_CACHE = {}


def _get_program():
    if "nc" not in _PROGRAM_CACHE:
        _PROGRAM_CACHE["nc"] = _build_program()
    return _PROGRAM_CACHE["nc"]


def kernel(mesh_signal, bary_coordinates, neighbor_weights, self_weights, bias, kernel,
           _trace=False, _core_ids=None):
    nc = _get_program()
    in_maps = _host_prep(
        mesh_signal, bary_coordinates, neighbor_weights, self_weights, bias, kernel
    )
    core_ids = list(range(NC)) if _core_ids is None else _core_ids
    res = run_bass_kernel_spmd(nc, in_maps[: len(core_ids)], core_ids, trace=_trace)
    out = np.zeros((1, N, O, T), dtype=np.float32)
    for i in range(len(core_ids)):
        o = res.results[i]["out"]              # [128, 2, VNC]
        out[0, i * VS : (i + 1) * VS] = (
            o[:, :, :VS].transpose(2, 1, 0).reshape(VS, O, T)
        )
    if _trace:
        globals()["kernel"]._last_exec_ns = res.exec_time_ns
    return out
